# revision 1
# baseline (speedup 1.0000x reference)
"""Trainium2 Bass kernel for a dense pre-norm transformer block.

Problem: x[8, 1024, 768]; per-batch-element transformer block
  (LN1 -> qkv -> 12-head attention -> proj residual -> LN2 -> MLP(gelu) residual).

Strategy:
  - Pure data-parallel: 8 NeuronCores, one batch element each. No collectives.
  - Activations are kept channel-major ("T layout", [C, tokens]) on device the
    whole time; the host transposes x / un-transposes the output (free wrt HW).
  - GEMMs run as float32r (full PE rate at moving-dim >= 256, near-fp32 bits).
    The residual stream and LN statistics stay full fp32.
  - LayerNorm stats via a ones-matmul, which broadcasts mean/E[x^2] to all 128
    partitions for free.
  - Attention: scores computed transposed ([key, query]) so softmax(attn) @ V
    needs no transposes; softmax denominators ride in a ones-column appended to
    V; per-head normalization is broadcast across partitions via a small DRAM
    round-trip, pipelined per head-pair.  Head pairs share the PE array via
    row-group tiling (K=64).  V and per-pair q/k production are interleaved
    with attention so exp (ScalarE) overlaps PE matmuls.
  - GELU (exact erf) fused into the fc1 PSUM eviction; residual adds fused into
    PSUM evictions via scalar_tensor_tensor.
"""

import ml_dtypes
import numpy as np

import concourse.bacc as bacc
import concourse.bass as bass
import concourse.mybir as mybir
from concourse import tile
from concourse.bass_utils import run_bass_kernel_spmd

AF = mybir.ActivationFunctionType
ALU = mybir.AluOpType
f32 = mybir.dt.float32
f32r = mybir.dt.float32r
bf16 = mybir.dt.bfloat16

P = 128
DIM = 768
CT = DIM // P            # 6 channel tiles
N = 1024                 # tokens
NT = N // P              # 8 token tiles
NH = 12                  # heads
DH = 64                  # head dim
HID = 3072
HT = HID // P            # 24 hidden tiles
B = 8
EPS = 1e-5
SCALE = DH ** -0.5

DEBUG_TAPS = False       # adds intermediate ExternalOutputs for debugging


def _t6(dram_2d):
    """View a [6*128, M] DRAM tensor/AP as [128, 6, M] (partition-major tiles)."""
    return dram_2d.rearrange("(a p) m -> p a m", p=P)


def build_nc(reps=1):
    nc = bacc.Bacc("TRN2", target_bir_lowering=False, debug=False)

    # ---- I/O ----
    xT = nc.dram_tensor("xT", [DIM, N], f32, kind="ExternalInput")
    wqk = nc.dram_tensor("wqk", [12, P, CT * P], bf16, kind="ExternalInput")
    wv = nc.dram_tensor("wv", [P, CT, DIM], bf16, kind="ExternalInput")
    wproj = nc.dram_tensor("wproj", [P, CT, DIM], bf16, kind="ExternalInput")
    wfc1 = nc.dram_tensor("wfc1", [HT, P, CT * P], bf16, kind="ExternalInput")
    wfc2 = nc.dram_tensor("wfc2", [HT, 2, P, 3 * P], bf16, kind="ExternalInput")
    bqk = nc.dram_tensor("bqk", [P, 12], f32, kind="ExternalInput")
    bv = nc.dram_tensor("bv", [DIM], f32, kind="ExternalInput")
    bproj = nc.dram_tensor("bproj", [P, CT], f32, kind="ExternalInput")
    bfc1 = nc.dram_tensor("bfc1", [P, HT], f32, kind="ExternalInput")
    bfc2 = nc.dram_tensor("bfc2", [P, CT], f32, kind="ExternalInput")
    g1 = nc.dram_tensor("g1", [P, CT], f32, kind="ExternalInput")
    b1 = nc.dram_tensor("b1", [P, CT], f32, kind="ExternalInput")
    g2 = nc.dram_tensor("g2", [P, CT], f32, kind="ExternalInput")
    b2 = nc.dram_tensor("b2", [P, CT], f32, kind="ExternalInput")
    ones_v = nc.dram_tensor("ones_v", [NT * NH], bf16, kind="ExternalInput")
    outT = nc.dram_tensor("outT", [DIM, N], f32, kind="ExternalOutput")

    taps = {}
    if DEBUG_TAPS:
        for name, shape, dt_ in [
            ("t_h1", [DIM, N], f32r), ("t_v", [P, NT, NH, DH + 1], f32r),
            ("t_o", [DIM, N], f32r), ("t_x2", [DIM, N], f32),
        ]:
            taps[name] = nc.dram_tensor(name, shape, dt_, kind="ExternalOutput")

    args = locals()
    with tile.TileContext(nc) as tc:
        _body(nc, tc, args, reps)
    nc.compile()
    return nc


def _body(nc, tc, t, reps=1):
    xT, outT = t["xT"], t["outT"]
    _dma_rr = [0]
    def dma_load(out, in_):
        eng = (nc.sync, nc.scalar, nc.gpsimd)[_dma_rr[0] % 3]
        _dma_rr[0] += 1
        eng.dma_start(out, in_)
    wqk, wv, wproj, wfc1, wfc2 = t["wqk"], t["wv"], t["wproj"], t["wfc1"], t["wfc2"]
    taps = t["taps"]

    with (
        tc.tile_pool(name="const", bufs=1) as const,
        tc.tile_pool(name="resid", bufs=1) as resid,
        tc.tile_pool(name="hpool", bufs=1) as hpool,
        tc.tile_pool(name="dram", bufs=1, space="DRAM") as dram,
    ):
        # ---- residual stream (channel-major, fp32) ----
        xsb = resid.tile([P, CT, N], f32)
        for ct in range(CT):
            dma_load(xsb[:, ct, :], xT[ct * P:(ct + 1) * P, :])

        # ---- constants ----
        ones_ln = const.tile([P, P], f32)
        nc.vector.memset(ones_ln[:], 1.0 / DIM)
        ones_r = const.tile([P, P], f32r)
        nc.scalar.copy(ones_r[:], ones_ln[:])
        eps_t = const.tile([P, 1], f32)
        nc.vector.memset(eps_t[:], EPS)
        bqk_sb = const.tile([P, 12], f32)
        nc.sync.dma_start(bqk_sb[:], t["bqk"][:])
        bproj_sb = const.tile([P, CT], f32)
        nc.sync.dma_start(bproj_sb[:], t["bproj"][:])
        bfc1_sb = const.tile([P, HT], f32)
        nc.sync.dma_start(bfc1_sb[:], t["bfc1"][:])
        bfc2_sb = const.tile([P, CT], f32)
        nc.sync.dma_start(bfc2_sb[:], t["bfc2"][:])
        g1_sb = const.tile([P, CT], f32)
        nc.sync.dma_start(g1_sb[:], t["g1"][:])
        b1_sb = const.tile([P, CT], f32)
        nc.sync.dma_start(b1_sb[:], t["b1"][:])
        g2_sb = const.tile([P, CT], f32)
        nc.sync.dma_start(g2_sb[:], t["g2"][:])
        b2_sb = const.tile([P, CT], f32)
        nc.sync.dma_start(b2_sb[:], t["b2"][:])
        # v-bias broadcast to all partitions
        vb_sb = const.tile([P, DIM], f32)
        bv_ap = t["bv"][:]
        bv_bcast = bass.AP(tensor=bv_ap.tensor, offset=bv_ap.offset,
                           ap=[[0, P], [1, DIM]])
        nc.gpsimd.dma_start(vb_sb[:], bv_bcast)

        def layer_norm_T(src, dst, g_sb, b_sb):
            """src: [P, CT, N] fp32; dst: [P, CT, N] f32r = LN(src) * g + b."""
            with (
                tc.tile_pool(name="ln_tmp", bufs=1) as tmp,
                tc.tile_pool(name="ln_ps", bufs=1, space="PSUM") as lps,
            ):
                mu_ps = lps.tile([P, N], f32)
                e2_ps = lps.tile([P, N], f32)
                for ct in range(CT):
                    xr = tmp.tile([P, N], f32r, tag="xr", bufs=2)
                    nc.vector.tensor_copy(xr[:], src[:, ct, :])
                    sq = tmp.tile([P, N], f32r, tag="sq", bufs=2)
                    nc.scalar.square(sq[:], src[:, ct, :])
                    for h in range(2):
                        sl = bass.ts(h, 512)
                        nc.tensor.matmul(
                            mu_ps[:, sl], ones_r[:], xr[:, sl],
                            start=(ct == 0), stop=(ct == CT - 1))
                        nc.tensor.matmul(
                            e2_ps[:, sl], ones_r[:], sq[:, sl],
                            start=(ct == 0), stop=(ct == CT - 1))
                mu_sb = tmp.tile([P, N], f32)
                nc.vector.tensor_copy(mu_sb[:], mu_ps[:])
                mu2 = tmp.tile([P, N], f32)
                nc.vector.tensor_mul(mu2[:], mu_sb[:], mu_sb[:])
                var = tmp.tile([P, N], f32)
                nc.vector.tensor_sub(var[:], e2_ps[:], mu2[:])
                sd = tmp.tile([P, N], f32)
                nc.scalar.activation(sd[:], var[:], AF.Sqrt, bias=eps_t[:], scale=1.0)
                rstd = tmp.tile([P, N], f32)
                nc.vector.reciprocal(rstd[:], sd[:])
                for ct in range(CT):
                    eng = nc.vector if ct < 4 else nc.gpsimd
                    t1 = tmp.tile([P, N], f32, tag="t1", bufs=4)
                    eng.tensor_sub(t1[:], src[:, ct, :], mu_sb[:])
                    eng.tensor_mul(t1[:], t1[:], rstd[:])
                    nc.scalar.activation(
                        dst[:, ct, :], t1[:], AF.Identity,
                        bias=b_sb[:, ct:ct + 1], scale=g_sb[:, ct:ct + 1])

        for _rep in range(reps):
            with (
                tc.tile_pool(name="qkv_w", bufs=1) as qw,
                tc.tile_pool(name="attn", bufs=1) as attn,
                tc.tile_pool(name="pj_w", bufs=1) as pw,
            ):
                vsb = attn.tile([P, NT, NH, DH + 1], bf16)
                osb = attn.tile([P, CT, N], bf16)
                wp_sb = pw.tile([P, CT, DIM], bf16)

                # ======== LN1 ========
                h1 = hpool.tile([P, CT, N], bf16, tag="h")
                layer_norm_T(xsb, h1, g1_sb, b1_sb)
                nc.scalar.dma_start(wp_sb[:], wproj[:])
                if taps:
                    nc.sync.dma_start(_t6(taps["t_h1"]), h1[:])

                dscr = dram.tile([NH, N], bf16, tag="dscr")
                # ==== merged QKV + attention, pipelined per head pair ====
                with (
                    tc.tile_pool(name="att_sb", bufs=1) as asb,
                    tc.tile_pool(name="att_ps", bufs=1, space="PSUM") as aps,
                ):
                    # ---- V (natural layout + ones column), needed by all pairs
                    ones_col = vsb[:, :, :, DH].rearrange("p a b -> p (a b)")
                    ov_ap = t["ones_v"][:]
                    nc.gpsimd.dma_start(
                        ones_col,
                        bass.AP(tensor=ov_ap.tensor, offset=ov_ap.offset,
                                ap=[[0, P], [1, NT * NH]]))
                    wv_sb = qw.tile([P, CT, DIM], bf16)
                    nc.scalar.dma_start(wv_sb[:], wv[:])
                    def v_tile(it):
                        vps = aps.tile([P, N], f32, tag="sc", bufs=2, name="vps")
                        for c0, cn in ((0, 512), (512, 256)):
                            for kt in range(CT):
                                nc.tensor.matmul(
                                    vps[:, c0:c0 + cn],
                                    h1[:, kt, it * P:(it + 1) * P],
                                    wv_sb[:, kt, c0:c0 + cn],
                                    start=(kt == 0), stop=(kt == CT - 1))
                        nc.vector.scalar_tensor_tensor(
                            out=vsb[:, it, :, 0:DH],
                            in0=vps[:, 0:DIM].rearrange("p (h d) -> p h d", d=DH),
                            scalar=0.0, op0=ALU.add,
                            in1=vb_sb[:].rearrange("p (h d) -> p h d", d=DH),
                            op1=ALU.add)

                    def qk_prod(tp):
                        qt = asb.tile([P, N], bf16, tag="qt", bufs=2, name="qt")
                        kt2 = asb.tile([P, N], bf16, tag="kt2", bufs=2,
                                       name="kt2")
                        for dst_sb, mt in ((qt, tp), (kt2, CT + tp)):
                            wt = qw.tile([P, CT, P], bf16, tag="wqk", bufs=2,
                                         name="wt")
                            dma_load(
                                wt[:].rearrange("p a m -> p (a m)"),
                                wqk[mt, :, :])
                            qkps = aps.tile([P, N], f32, tag="sc", bufs=2,
                                            name="qkps")
                            for h in range(2):
                                sl = bass.ts(h, 512)
                                for kt in range(CT):
                                    nc.tensor.matmul(
                                        qkps[:, sl], wt[:, kt, :], h1[:, kt, sl],
                                        start=(kt == 0), stop=(kt == CT - 1))
                            nc.vector.tensor_scalar(
                                out=dst_sb[:], in0=qkps[:],
                                scalar1=bqk_sb[:, mt:mt + 1], scalar2=None,
                                op0=ALU.add)
                        return qt, kt2

                    def attn_jt(tp, jt, qt, kt2, av0, av1):
                        sc0 = aps.tile([P, N], f32, tag="sc", bufs=2,
                                       name="sc0")
                        sc1 = aps.tile([P, N], f32, tag="sc", bufs=2,
                                       name="sc1")
                        js = slice(jt * P, (jt + 1) * P)
                        for h in range(2):
                            sl = bass.ts(h, 512)
                            nc.tensor.matmul(
                                sc0[:, sl], kt2[0:DH, js],
                                qt[0:DH, sl], tile_position=(0, 0))
                            nc.tensor.matmul(
                                sc1[:, sl], kt2[DH:P, js],
                                qt[DH:P, sl], tile_position=(DH, 0))
                        e0 = asb.tile([P, N], bf16, tag="exp", bufs=4,
                                      name="e0")
                        nc.scalar.activation(e0[:], sc0[:], AF.Exp, scale=SCALE)
                        e1 = asb.tile([P, N], bf16, tag="exp", bufs=4,
                                      name="e1")
                        nc.scalar.activation(e1[:], sc1[:], AF.Exp, scale=SCALE)
                        for h in range(2):
                            sl = bass.ts(h, 512)
                            nc.tensor.matmul(
                                av0[:, sl], vsb[:, jt, 2 * tp, :], e0[:, sl],
                                start=(jt == 0), stop=(jt == NT - 1))
                            nc.tensor.matmul(
                                av1[:, sl], vsb[:, jt, 2 * tp + 1, :],
                                e1[:, sl],
                                start=(jt == 0), stop=(jt == NT - 1))

                    def finish_pair(tp, av0, av1):
                        # evict unnormalized o^T (DVE) and denominators -> DRAM
                        nc.vector.tensor_copy(osb[0:DH, tp, :], av0[0:DH, :])
                        te = asb.tile([DH + 1, N], bf16, tag="tmpo", bufs=2,
                                      name="te")
                        nc.vector.tensor_copy(te[DH:DH + 1, :], av0[DH:DH + 1, :])
                        nc.sync.dma_start(dscr[2 * tp, :], te[DH:DH + 1, :])
                        to = asb.tile([DH + 1, N], bf16, tag="tmpo", bufs=2,
                                      name="to")
                        nc.vector.tensor_copy(to[:], av1[:])
                        nc.sync.dma_start(osb[DH:P, tp, :], to[0:DH, :])
                        nc.sync.dma_start(dscr[2 * tp + 1, :], to[DH:DH + 1, :])
                        # normalize: Rt = 1/denoms broadcast across partitions
                        Rt = asb.tile([P, N], bf16, tag="Rt", bufs=2, name="Rt")
                        for hh in range(2):
                            srcb = bass.AP(
                                tensor=dscr.tensor,
                                offset=dscr.offset + (2 * tp + hh) * N,
                                ap=[[0, DH], [1, N]])
                            nc.gpsimd.dma_start(Rt[hh * DH:(hh + 1) * DH, :], srcb)
                        with nc.allow_low_precision(reason="f32r softmax denom"):
                            nc.vector.reciprocal(Rt[:], Rt[:])
                        nc.vector.tensor_mul(osb[:, tp, :], osb[:, tp, :], Rt[:])

                    # pair 0 interleaved with V production
                    qt0, kt20 = qk_prod(0)
                    av0 = aps.tile([DH + 1, N], f32, tag="av", bufs=2,
                                   name="av0")
                    av1 = aps.tile([DH + 1, N], f32, tag="av", bufs=2,
                                   name="av1")
                    for jt in range(NT):
                        v_tile(jt)
                        attn_jt(0, jt, qt0, kt20, av0, av1)
                    finish_pair(0, av0, av1)
                    for tp in range(1, CT):
                        qt, kt2 = qk_prod(tp)
                        av0 = aps.tile([DH + 1, N], f32, tag="av", bufs=2,
                                       name="av0")
                        av1 = aps.tile([DH + 1, N], f32, tag="av", bufs=2,
                                       name="av1")
                        for jt in range(NT):
                            attn_jt(tp, jt, qt, kt2, av0, av1)
                        finish_pair(tp, av0, av1)
                if taps:
                    nc.sync.dma_start(taps["t_v"][:], vsb[:])
                    nc.sync.dma_start(_t6(taps["t_o"]), osb[:])

                # ======== proj + residual ========
                with (
                    tc.tile_pool(name="pj_ps", bufs=1, space="PSUM") as pps,
                ):
                    for mt in range(CT):
                        for h in range(2):
                            sl = bass.ts(h, 512)
                            ps = pps.tile([P, 512], f32, tag="ps", bufs=6,
                                          name="ps")
                            for kt in range(CT):
                                nc.tensor.matmul(
                                    ps[:], wp_sb[:, kt, mt * P:(mt + 1) * P],
                                    osb[:, kt, sl],
                                    start=(kt == 0), stop=(kt == CT - 1))
                            nc.vector.scalar_tensor_tensor(
                                out=xsb[:, mt, sl], in0=ps[:],
                                scalar=bproj_sb[:, mt:mt + 1], op0=ALU.add,
                                in1=xsb[:, mt, sl], op1=ALU.add)
            # attention pools released here
            if taps:
                nc.sync.dma_start(_t6(taps["t_x2"]), xsb[:])

            # ======== LN2 + MLP ========
            # fc1 streamed once into a full gelu(h3) SBUF buffer; fc2 in two
            # output-group passes over it.  Halves the MLP weight traffic.
            with tc.tile_pool(name="mlp_w", bufs=1) as mw:
                h2 = hpool.tile([P, CT, N], bf16, tag="h")
                layer_norm_T(xsb, h2, g2_sb, b2_sb)
                h3sb = mw.tile([P, HT, N], bf16, name="h3sb")
                with tc.tile_pool(name="fc1_ps", bufs=1, space="PSUM") as f1p:
                    for ct in range(HT):
                        w1t = mw.tile([P, CT, P], bf16, tag="w1t", bufs=3,
                                      name="w1t")
                        dma_load(
                            w1t[:].rearrange("p a m -> p (a m)"),
                            wfc1[ct, :, :])
                        h3ps = f1p.tile([P, N], f32, tag="h3ps", bufs=3,
                                        name="h3ps")
                        for h in range(2):
                            sl = bass.ts(h, 512)
                            for kt in range(CT):
                                nc.tensor.matmul(
                                    h3ps[:, sl], w1t[:, kt, :], h2[:, kt, sl],
                                    start=(kt == 0), stop=(kt == CT - 1))
                        nc.scalar.activation(
                            h3sb[:, ct, :], h3ps[:], AF.Gelu,
                            bias=bfc1_sb[:, ct:ct + 1], scale=1.0)
                with tc.tile_pool(name="fc2_ps", bufs=1, space="PSUM") as f2p:
                    for g in range(2):  # output groups: mt 0-2, 3-5
                        f2ps = [f2p.tile([P, N], f32, tag=f"f2_{i}", bufs=1,
                                         name=f"f2ps{i}") for i in range(3)]
                        for ct in range(HT):
                            w2t = mw.tile([P, 3 * P], bf16, tag="w2t", bufs=3,
                                          name="w2t")
                            dma_load(w2t[:], wfc2[ct, g, :, :])
                            for i in range(3):
                                for h in range(2):
                                    sl = bass.ts(h, 512)
                                    nc.tensor.matmul(
                                        f2ps[i][:, sl],
                                        w2t[:, i * P:(i + 1) * P],
                                        h3sb[:, ct, sl],
                                        start=(ct == 0), stop=(ct == HT - 1))
                        for i in range(3):
                            mt = g * 3 + i
                            nc.vector.scalar_tensor_tensor(
                                out=xsb[:, mt, :], in0=f2ps[i][:],
                                scalar=bfc2_sb[:, mt:mt + 1], op0=ALU.add,
                                in1=xsb[:, mt, :], op1=ALU.add)
                            if _rep == reps - 1:
                                nc.sync.dma_start(
                                    _t6(outT)[:, mt, :], xsb[:, mt, :])


_NC_CACHE = None


def _get_nc():
    global _NC_CACHE
    if _NC_CACHE is None:
        _NC_CACHE = build_nc()
    return _NC_CACHE


def _prep_shared(qkv_w, qkv_b, proj_w, proj_b, fc1_w, fc1_b, fc2_w, fc2_b,
                 ln1_g, ln1_b, ln2_g, ln2_b):
    c = lambda a: np.ascontiguousarray(np.asarray(a, dtype=np.float32))
    return {
        "wqk": np.ascontiguousarray(np.asarray(qkv_w, np.float32)[:, :2 * DIM].reshape(CT, P, 12, P).transpose(2, 1, 0, 3).reshape(12, P, CT * P)).astype(ml_dtypes.bfloat16),
        "wv": np.ascontiguousarray(np.asarray(qkv_w, np.float32)[:, 2 * DIM:].reshape(CT, P, DIM).transpose(1, 0, 2)).astype(ml_dtypes.bfloat16),
        "wproj": np.ascontiguousarray(np.asarray(proj_w, np.float32).reshape(CT, P, DIM).transpose(1, 0, 2)).astype(ml_dtypes.bfloat16),
        "wfc1": np.ascontiguousarray(np.asarray(fc1_w, np.float32).reshape(CT, P, HT, P).transpose(2, 1, 0, 3).reshape(HT, P, CT * P)).astype(ml_dtypes.bfloat16),
        "wfc2": np.ascontiguousarray(np.asarray(fc2_w, np.float32).reshape(HT, P, 2, 3 * P).transpose(0, 2, 1, 3)).astype(ml_dtypes.bfloat16),
        "bqk": c(np.asarray(qkv_b)[:2 * DIM].reshape(12, P).T),
        "bv": c(np.asarray(qkv_b)[2 * DIM:]),
        "bproj": c(np.asarray(proj_b).reshape(CT, P).T),
        "bfc1": c(np.asarray(fc1_b).reshape(HT, P).T),
        "bfc2": c(np.asarray(fc2_b).reshape(CT, P).T),
        "g1": c(np.asarray(ln1_g).reshape(CT, P).T),
        "b1": c(np.asarray(ln1_b).reshape(CT, P).T),
        "g2": c(np.asarray(ln2_g).reshape(CT, P).T),
        "b2": c(np.asarray(ln2_b).reshape(CT, P).T),
        "ones_v": np.ones(NT * NH, ml_dtypes.bfloat16),
    }


def run(x, shared, **spmd_kwargs):
    nc = _get_nc()
    x = np.asarray(x, dtype=np.float32)
    in_maps = [
        {**shared, "xT": np.ascontiguousarray(x[b].T)} for b in range(B)
    ]
    res = run_bass_kernel_spmd(nc, in_maps, core_ids=list(range(B)), **spmd_kwargs)
    out = np.stack([res.results[b]["outT"].T for b in range(B)])
    return out.astype(np.float32), res


def kernel(x, ln1_g, ln1_b, qkv_w, qkv_b, proj_w, proj_b,
           ln2_g, ln2_b, fc1_w, fc1_b, fc2_w, fc2_b):
    shared = _prep_shared(qkv_w, qkv_b, proj_w, proj_b, fc1_w, fc1_b,
                          fc2_w, fc2_b, ln1_g, ln1_b, ln2_g, ln2_b)
    out, _ = run(x, shared)
    return out



# revision 5
# speedup vs baseline: 1.6499x; 1.6499x over previous
"""Trainium2 Bass kernel for a dense pre-norm transformer block.

Problem: x[8, 1024, 768]; per-batch-element transformer block
  (LN1 -> qkv -> 12-head attention -> proj residual -> LN2 -> MLP(gelu) residual).

Strategy (v2, fp8):
  - Pure data-parallel: 8 NeuronCores, one batch element each. No collectives.
  - Activations channel-major ("T layout", [C, tokens]); host transposes.
  - fp8e4(+DoubleRow, 2 k-tiles per matmul) for qkv/V/AV/proj/fc2 GEMMs;
    fc1 stays bf16 (dominant error site); scores bf16 with PE row-group
    concurrency; residual stream + LN stats fp32/f32r.
  - All weights SBUF-resident (~72KB/partition), loaded once, zero
    steady-state weight DMA.
  - LN gains folded into the following weights, LN biases folded into the
    following biases (host-side). Normalize = sub+mul only. rstd computed as
    exp(-0.5*ln(var+eps)) so ACT stays on the ln/exp table set through
    LN+attention; token-half pipelining hides the stat chain.
  - Softmax denominators ride a ones-column in V (also fp8/DoubleRow);
    per-head-pair normalization broadcast via small DRAM round-trip.
"""

import ml_dtypes
import numpy as np

import concourse.bacc as bacc
import concourse.bass as bass
import concourse.mybir as mybir
from concourse import tile
from concourse.bass_utils import run_bass_kernel_spmd

AF = mybir.ActivationFunctionType
ALU = mybir.AluOpType
PM = mybir.MatmulPerfMode
f32 = mybir.dt.float32
f32r = mybir.dt.float32r
bf16 = mybir.dt.bfloat16
f8 = mybir.dt.float8e4

P = 128
DIM = 768
CT = DIM // P            # 6 channel tiles
KP = CT // 2             # 3 channel-tile pairs (DoubleRow)
N = 1024                 # tokens
NT = N // P              # 8 token tiles
JP = NT // 2             # 4 key-tile pairs
NH = 12                  # heads
DH = 64                  # head dim
VW = 80                  # padded V row width (DH + ones col, 16B aligned)
HID = 3072
HT = HID // P            # 24 hidden tiles
HP = HT // 2             # 12 hidden-tile pairs
B = 8
EPS = 1e-5
SCALE = DH ** -0.5
SW = 16.0                # fp8 weight scale (qkv/v/fc2)
ISW = 1.0 / SW


def _t6(dram_2d):
    return dram_2d.rearrange("(a p) m -> p a m", p=P)


def build_nc(reps=1):
    nc = bacc.Bacc("TRN2", target_bir_lowering=False, debug=False)

    # ---- I/O ----
    xT = nc.dram_tensor("xT", [DIM, N], f32r, kind="ExternalInput")
    wqk = nc.dram_tensor("wqk", [P, 12 * KP * 2 * P], f8, kind="ExternalInput")
    wv = nc.dram_tensor("wv", [P, KP * 2 * DIM], f8, kind="ExternalInput")
    wproj = nc.dram_tensor("wproj", [P, KP * 2 * DIM], f8, kind="ExternalInput")
    wfc1 = nc.dram_tensor("wfc1", [P, HT * CT * P], bf16, kind="ExternalInput")
    wfc2 = nc.dram_tensor("wfc2", [P, HP * 2 * DIM], f8, kind="ExternalInput")
    bqk = nc.dram_tensor("bqk", [P, 12], f32, kind="ExternalInput")
    bv = nc.dram_tensor("bv", [DIM], f32, kind="ExternalInput")
    bproj = nc.dram_tensor("bproj", [P, CT], f32, kind="ExternalInput")
    bfc1 = nc.dram_tensor("bfc1", [P, HT], f32, kind="ExternalInput")
    bfc2 = nc.dram_tensor("bfc2", [P, CT], f32, kind="ExternalInput")
    outT = nc.dram_tensor("outT", [DIM, N], f32r, kind="ExternalOutput")

    args = locals()
    with tile.TileContext(nc) as tc:
        _body(nc, tc, args, reps)
    nc.compile()
    return nc


def _body(nc, tc, t, reps=1):
    xT, outT = t["xT"], t["outT"]

    with (
        tc.tile_pool(name="const", bufs=1) as const,
        tc.tile_pool(name="work", bufs=1) as work,
        tc.tile_pool(name="dram", bufs=1, space="DRAM") as dram,
    ):
        # ---- SBUF-resident weights (loaded once) ----
        wqk_sb = const.tile([P, 12, KP, 2, P], f8)
        nc.sync.dma_start(wqk_sb[:].rearrange("p a b c d -> p (a b c d)"),
                          t["wqk"][:])
        wv_sb = const.tile([P, KP, 2, DIM], f8)
        nc.sync.dma_start(wv_sb[:].rearrange("p a b c -> p (a b c)"),
                          t["wv"][:])
        wp_sb = const.tile([P, KP, 2, DIM], f8)
        nc.sync.dma_start(wp_sb[:].rearrange("p a b c -> p (a b c)"),
                          t["wproj"][:])
        w1_sb = const.tile([P, HT, CT, P], bf16)
        nc.scalar.dma_start(w1_sb[:].rearrange("p a b c -> p (a b c)"),
                            t["wfc1"][:])
        w2_sb = const.tile([P, HP, 2, DIM], f8)
        nc.sync.dma_start(w2_sb[:].rearrange("p a b c -> p (a b c)"),
                          t["wfc2"][:])

        # ---- constants ----
        ones_ln = const.tile([P, P], f32)
        nc.vector.memset(ones_ln[:], 1.0 / DIM)
        ones_r = const.tile([P, P], f32r)
        nc.scalar.copy(ones_r[:], ones_ln[:])
        eps_t = const.tile([P, 1], f32)
        nc.vector.memset(eps_t[:], EPS)
        bqk_sb = const.tile([P, 12], f32)
        nc.sync.dma_start(bqk_sb[:], t["bqk"][:])
        bproj_sb = const.tile([P, CT], f32)
        nc.sync.dma_start(bproj_sb[:], t["bproj"][:])
        bfc1_sb = const.tile([P, HT], f32)
        nc.sync.dma_start(bfc1_sb[:], t["bfc1"][:])
        bfc2_sb = const.tile([P, CT], f32)
        nc.sync.dma_start(bfc2_sb[:], t["bfc2"][:])
        vb_sb = const.tile([P, DIM], f32)
        bv_ap = t["bv"][:]
        nc.gpsimd.dma_start(
            vb_sb[:],
            bass.AP(tensor=bv_ap.tensor, offset=bv_ap.offset,
                    ap=[[0, P], [1, DIM]]))

        # ---- persistent activations ----
        xsb = const.tile([P, CT, N], f32r)        # residual stream
        for ct in range(CT):
            nc.sync.dma_start(xsb[:, ct, :], xT[ct * P:(ct + 1) * P, :])
        h1 = const.tile([P, CT, N], f8)           # LN1 out
        h2 = const.tile([P, CT, N], bf16)         # LN2 out
        h3 = const.tile([P, HT, N], f8)           # gelu(fc1) out
        ob = const.tile([P, CT, N], bf16)         # unnormalized attn out
        o8 = const.tile([P, CT, N], f8)           # normalized attn out
        vsb = const.tile([P, NH, JP, 2, VW], f8)  # V + ones col, padded
        with nc.allow_low_precision(reason="ones column exact in fp8"):
            nc.vector.memset(vsb[:, :, :, :, DH:DH + 1], 1.0)
        dscr = dram.tile([NH, N], bf16)

        def layer_norm_T(src, dst):
            """dst = (src - mu) * rstd, per token-half; dst is fp8/bf16."""
            with (
                tc.tile_pool(name="ln_tmp", bufs=1) as tmp,
                tc.tile_pool(name="ln_ps", bufs=1, space="PSUM") as lps,
            ):
                for hh in range(2):
                    sl = bass.ts(hh, 512)
                    mu_ps = lps.tile([P, 512], f32, tag="mups", bufs=2,
                                     name="mu_ps")
                    e2_ps = lps.tile([P, 512], f32, tag="e2ps", bufs=2,
                                     name="e2_ps")
                    for ct in range(CT):
                        sq = tmp.tile([P, 512], f32r, tag="sq", bufs=3,
                                      name="sq")
                        eng = nc.gpsimd if ct % 2 else nc.vector
                        eng.tensor_mul(sq[:], src[:, ct, sl], src[:, ct, sl])
                        nc.tensor.matmul(
                            mu_ps[:], ones_r[:], src[:, ct, sl],
                            start=(ct == 0), stop=(ct == CT - 1))
                        nc.tensor.matmul(
                            e2_ps[:], ones_r[:], sq[:],
                            start=(ct == 0), stop=(ct == CT - 1))
                    mu_sb = tmp.tile([P, 512], f32, tag="musb", bufs=2,
                                     name="mu_sb")
                    nc.vector.tensor_copy(mu_sb[:], mu_ps[:])
                    mu2 = tmp.tile([P, 512], f32, tag="mu2", bufs=2,
                                   name="mu2")
                    nc.vector.tensor_mul(mu2[:], mu_sb[:], mu_sb[:])
                    var = tmp.tile([P, 512], f32, tag="var", bufs=2,
                                   name="var")
                    nc.vector.tensor_sub(var[:], e2_ps[:], mu2[:])
                    # rstd = exp(-0.5 * ln(var + eps)): stays on act set 6
                    lnv = tmp.tile([P, 512], f32, tag="lnv", bufs=2,
                                   name="lnv")
                    nc.scalar.activation(lnv[:], var[:], AF.Ln,
                                         bias=eps_t[:], scale=1.0)
                    rstd = tmp.tile([P, 512], f32, tag="rstd", bufs=2,
                                    name="rstd")
                    nc.scalar.activation(rstd[:], lnv[:], AF.Exp, scale=-0.5)
                    for ct in range(CT):
                        eng = nc.gpsimd if ct >= 4 else nc.vector
                        t1 = tmp.tile([P, 512], f32, tag="t1", bufs=4,
                                      name="t1")
                        eng.tensor_sub(t1[:], src[:, ct, sl], mu_sb[:])
                        with nc.allow_low_precision(reason="ln out fp8"):
                            eng.tensor_mul(dst[:, ct, sl], t1[:], rstd[:])

        for _rep in range(reps):
            # ======== LN1 ========
            layer_norm_T(xsb, h1)

            # ==== merged QKV + attention, pipelined per head pair ====
            with (
                tc.tile_pool(name="att_sb", bufs=1) as asb,
                tc.tile_pool(name="att_ps", bufs=1, space="PSUM") as aps,
            ):
                def v_tile(it):
                    """V for token-tile it (tokens on partitions)."""
                    vps = aps.tile([P, DIM], f32, tag="sc", bufs=2,
                                   name="vps")
                    for c0, cn in ((0, 384), (384, 384)):
                        for kp in range(KP):
                            nc.tensor.matmul(
                                vps[:, c0:c0 + cn],
                                h1[:, 2 * kp:2 * kp + 2,
                                   it * P:(it + 1) * P],
                                wv_sb[:, kp, :, c0:c0 + cn],
                                start=(kp == 0), stop=(kp == KP - 1),
                                perf_mode=PM.DoubleRow)
                    with nc.allow_low_precision(reason="v fp8"):
                        nc.vector.scalar_tensor_tensor(
                            out=vsb[:, :, it // 2, it % 2, 0:DH],
                            in0=vps[:].rearrange("p (h d) -> p h d", d=DH),
                            scalar=ISW, op0=ALU.mult,
                            in1=vb_sb[:].rearrange("p (h d) -> p h d", d=DH),
                            op1=ALU.add)

                def qk_prod(tp):
                    qt = asb.tile([P, N], bf16, tag="qt", bufs=2, name="qt")
                    kt2 = asb.tile([P, N], bf16, tag="kt2", bufs=2,
                                   name="kt2")
                    for dst_sb, mt in ((qt, tp), (kt2, CT + tp)):
                        qkps = aps.tile([P, N], f32, tag="sc", bufs=2,
                                        name="qkps")
                        for hh in range(2):
                            sl = bass.ts(hh, 512)
                            for kp in range(KP):
                                nc.tensor.matmul(
                                    qkps[:, sl],
                                    wqk_sb[:, mt, kp, :, :],
                                    h1[:, 2 * kp:2 * kp + 2, sl],
                                    start=(kp == 0), stop=(kp == KP - 1),
                                    perf_mode=PM.DoubleRow)
                        nc.vector.tensor_scalar(
                            out=dst_sb[:], in0=qkps[:],
                            scalar1=ISW, scalar2=bqk_sb[:, mt:mt + 1],
                            op0=ALU.mult, op1=ALU.add)
                    return qt, kt2

                def attn_jp(tp, jp, qt, kt2, av0, av1, with_v):
                    """Scores+exp for key tiles 2jp,2jp+1; AV via DoubleRow."""
                    e0 = asb.tile([P, 2, N], f8, tag="e0", bufs=2, name="e0")
                    e1 = asb.tile([P, 2, N], f8, tag="e1", bufs=2, name="e1")
                    for j2 in range(2):
                        jt = 2 * jp + j2
                        if with_v:
                            v_tile(jt)
                        sc0 = aps.tile([P, N], f32, tag="sc", bufs=2,
                                       name="sc0")
                        sc1 = aps.tile([P, N], f32, tag="sc", bufs=2,
                                       name="sc1")
                        js = slice(jt * P, (jt + 1) * P)
                        for hh in range(2):
                            sl = bass.ts(hh, 512)
                            nc.tensor.matmul(
                                sc0[:, sl], kt2[0:DH, js],
                                qt[0:DH, sl], tile_position=(0, 0))
                            nc.tensor.matmul(
                                sc1[:, sl], kt2[DH:P, js],
                                qt[DH:P, sl], tile_position=(DH, 0))
                        with nc.allow_low_precision(reason="exp fp8"):
                            nc.scalar.activation(e0[:, j2, :], sc0[:],
                                                 AF.Exp, scale=SCALE)
                            nc.scalar.activation(e1[:, j2, :], sc1[:],
                                                 AF.Exp, scale=SCALE)
                    for hh in range(2):
                        sl = bass.ts(hh, 512)
                        nc.tensor.matmul(
                            av0[:, sl], vsb[:, 2 * tp, jp, :, 0:DH + 1],
                            e0[:, :, sl],
                            start=(jp == 0), stop=(jp == JP - 1),
                            perf_mode=PM.DoubleRow)
                        nc.tensor.matmul(
                            av1[:, sl], vsb[:, 2 * tp + 1, jp, :, 0:DH + 1],
                            e1[:, :, sl],
                            start=(jp == 0), stop=(jp == JP - 1),
                            perf_mode=PM.DoubleRow)

                def finish_pair(tp, av0, av1):
                    # evict unnormalized o^T + denominators -> DRAM
                    nc.vector.tensor_copy(ob[0:DH, tp, :], av0[0:DH, :])
                    te = asb.tile([DH + 1, N], bf16, tag="tmpo", bufs=2,
                                  name="te")
                    nc.vector.tensor_copy(te[DH:DH + 1, :],
                                          av0[DH:DH + 1, :])
                    nc.sync.dma_start(dscr[2 * tp, :], te[DH:DH + 1, :])
                    to = asb.tile([DH + 1, N], bf16, tag="tmpo", bufs=2,
                                  name="to")
                    nc.vector.tensor_copy(to[:], av1[:])
                    nc.sync.dma_start(ob[DH:P, tp, :], to[0:DH, :])
                    nc.sync.dma_start(dscr[2 * tp + 1, :], to[DH:DH + 1, :])
                    # normalize: Rt = 1/denoms broadcast across partitions
                    Rt = asb.tile([P, N], bf16, tag="Rt", bufs=2, name="Rt")
                    for hh in range(2):
                        srcb = bass.AP(
                            tensor=dscr.tensor,
                            offset=dscr.offset + (2 * tp + hh) * N,
                            ap=[[0, DH], [1, N]])
                        nc.gpsimd.dma_start(Rt[hh * DH:(hh + 1) * DH, :],
                                            srcb)
                    with nc.allow_low_precision(reason="softmax denom"):
                        nc.vector.reciprocal(Rt[:], Rt[:])
                        nc.vector.tensor_mul(o8[:, tp, :], ob[:, tp, :],
                                             Rt[:])

                # pair 0 interleaved with V production
                qt0, kt20 = qk_prod(0)
                av0 = aps.tile([DH + 1, N], f32, tag="av", bufs=2,
                               name="av0")
                av1 = aps.tile([DH + 1, N], f32, tag="av", bufs=2,
                               name="av1")
                for jp in range(JP):
                    attn_jp(0, jp, qt0, kt20, av0, av1, with_v=True)
                finish_pair(0, av0, av1)
                for tp in range(1, CT):
                    qt, kt2 = qk_prod(tp)
                    av0 = aps.tile([DH + 1, N], f32, tag="av", bufs=2,
                                   name="av0")
                    av1 = aps.tile([DH + 1, N], f32, tag="av", bufs=2,
                                   name="av1")
                    for jp in range(JP):
                        attn_jp(tp, jp, qt, kt2, av0, av1, with_v=False)
                    finish_pair(tp, av0, av1)

            # ======== proj + residual ========
            with tc.tile_pool(name="pj_ps", bufs=1, space="PSUM") as pps:
                for mt in range(CT):
                    for hh in range(2):
                        sl = bass.ts(hh, 512)
                        ps = pps.tile([P, 512], f32, tag="ps", bufs=6,
                                      name="ps")
                        for kp in range(KP):
                            nc.tensor.matmul(
                                ps[:],
                                wp_sb[:, kp, :, mt * P:(mt + 1) * P],
                                o8[:, 2 * kp:2 * kp + 2, sl],
                                start=(kp == 0), stop=(kp == KP - 1),
                                perf_mode=PM.DoubleRow)
                        nc.vector.scalar_tensor_tensor(
                            out=xsb[:, mt, sl], in0=ps[:],
                            scalar=bproj_sb[:, mt:mt + 1], op0=ALU.add,
                            in1=xsb[:, mt, sl], op1=ALU.add)

            # ======== LN2 + MLP ========
            layer_norm_T(xsb, h2)
            with tc.tile_pool(name="f1_ps", bufs=1, space="PSUM") as f1p:
                for ct in range(HT):
                    for hh in range(2):
                        sl = bass.ts(hh, 512)
                        h3ps = f1p.tile([P, 512], f32, tag="h3ps", bufs=6,
                                        name="h3ps")
                        for kt in range(CT):
                            nc.tensor.matmul(
                                h3ps[:], w1_sb[:, ct, kt, :],
                                h2[:, kt, sl],
                                start=(kt == 0), stop=(kt == CT - 1))
                        with nc.allow_low_precision(reason="h3 fp8"):
                            nc.scalar.activation(
                                h3[:, ct, sl], h3ps[:], AF.Gelu,
                                bias=bfc1_sb[:, ct:ct + 1], scale=1.0)
            with (
                tc.tile_pool(name="f2_ps", bufs=1, space="PSUM") as f2p,
                tc.tile_pool(name="f2_sb", bufs=1) as f2s,
            ):
                for g in range(2):  # output groups: mt 0-2, 3-5
                    f2ps = [f2p.tile([P, N], f32, tag=f"f2_{i}", bufs=1,
                                     name=f"f2ps{i}") for i in range(3)]
                    for hp in range(HP):
                        for i in range(3):
                            mc = g * 384 + i * P
                            for hh in range(2):
                                sl = bass.ts(hh, 512)
                                nc.tensor.matmul(
                                    f2ps[i][:, sl],
                                    w2_sb[:, hp, :, mc:mc + P],
                                    h3[:, 2 * hp:2 * hp + 2, sl],
                                    start=(hp == 0), stop=(hp == HP - 1),
                                    perf_mode=PM.DoubleRow)
                    for i in range(3):
                        mt = g * 3 + i
                        # t = psum/SW + bfc2 (ACT), then xsb += t (DVE)
                        tt = f2s.tile([P, N], f32, tag="tt", bufs=3,
                                      name="tt")
                        nc.scalar.activation(
                            tt[:], f2ps[i][:], AF.Identity,
                            bias=bfc2_sb[:, mt:mt + 1], scale=ISW)
                        nc.vector.tensor_add(xsb[:, mt, :], xsb[:, mt, :],
                                             tt[:])
                        if _rep == reps - 1:
                            nc.sync.dma_start(
                                _t6(outT)[:, mt, :], xsb[:, mt, :])


_NC_CACHE = None


def _get_nc():
    global _NC_CACHE
    if _NC_CACHE is None:
        _NC_CACHE = build_nc()
    return _NC_CACHE


def _prep_shared(qkv_w, qkv_b, proj_w, proj_b, fc1_w, fc1_b, fc2_w, fc2_b,
                 ln1_g, ln1_b, ln2_g, ln2_b):
    c = lambda a: np.ascontiguousarray(np.asarray(a, dtype=np.float32))
    f8np = ml_dtypes.float8_e4m3
    qkv_w = np.asarray(qkv_w, np.float32)
    fc1_w = np.asarray(fc1_w, np.float32)
    fc2_w = np.asarray(fc2_w, np.float32)
    proj_w = np.asarray(proj_w, np.float32)
    # fold LN gains into weights, LN biases into following biases
    wqkv_g = np.asarray(ln1_g, np.float32)[:, None] * qkv_w
    bqkv_eff = np.asarray(qkv_b, np.float32) + \
        np.asarray(ln1_b, np.float32) @ qkv_w
    wfc1_g = np.asarray(ln2_g, np.float32)[:, None] * fc1_w
    bfc1_eff = np.asarray(fc1_b, np.float32) + \
        np.asarray(ln2_b, np.float32) @ fc1_w

    # wqk: [P, 12, KP, 2, P]; m-tiles 0-5 = q, 6-11 = k; SW-scaled fp8
    wqk_h = (wqkv_g[:, :2 * DIM] * SW).reshape(KP, 2, P, 12, P)
    wqk_h = wqk_h.transpose(2, 3, 0, 1, 4).reshape(P, 12 * KP * 2 * P)
    # wv: [P, KP, 2, DIM]
    wv_h = (wqkv_g[:, 2 * DIM:] * SW).reshape(KP, 2, P, DIM)
    wv_h = wv_h.transpose(2, 0, 1, 3).reshape(P, KP * 2 * DIM)
    # wproj: [P, KP, 2, DIM] (unscaled)
    wp_h = proj_w.reshape(KP, 2, P, DIM).transpose(2, 0, 1, 3).reshape(
        P, KP * 2 * DIM)
    # wfc1: [P, HT, CT, P] bf16 (g2-folded)
    w1_h = wfc1_g.reshape(CT, P, HT, P).transpose(1, 2, 0, 3).reshape(
        P, HT * CT * P)
    # wfc2: [P, HP, 2, DIM] fp8, SW-scaled
    w2_h = (fc2_w * SW).reshape(HP, 2, P, DIM).transpose(2, 0, 1, 3).reshape(
        P, HP * 2 * DIM)
    return {
        "wqk": np.ascontiguousarray(wqk_h).astype(f8np),
        "wv": np.ascontiguousarray(wv_h).astype(f8np),
        "wproj": np.ascontiguousarray(wp_h).astype(f8np),
        "wfc1": np.ascontiguousarray(w1_h).astype(ml_dtypes.bfloat16),
        "wfc2": np.ascontiguousarray(w2_h).astype(f8np),
        "bqk": c(bqkv_eff[:2 * DIM].reshape(12, P).T),
        "bv": c(bqkv_eff[2 * DIM:]),
        "bproj": c(np.asarray(proj_b).reshape(CT, P).T),
        "bfc1": c(bfc1_eff.reshape(HT, P).T),
        "bfc2": c(np.asarray(fc2_b).reshape(CT, P).T),
    }


def run(x, shared, **spmd_kwargs):
    nc = _get_nc()
    x = np.asarray(x, dtype=np.float32)
    in_maps = [
        {**shared, "xT": np.ascontiguousarray(x[b].T)} for b in range(B)
    ]
    res = run_bass_kernel_spmd(nc, in_maps, core_ids=list(range(B)),
                               **spmd_kwargs)
    out = np.stack([res.results[b]["outT"].T for b in range(B)])
    return out.astype(np.float32), res


def kernel(x, ln1_g, ln1_b, qkv_w, qkv_b, proj_w, proj_b,
           ln2_g, ln2_b, fc1_w, fc1_b, fc2_w, fc2_b):
    shared = _prep_shared(qkv_w, qkv_b, proj_w, proj_b, fc1_w, fc1_b,
                          fc2_w, fc2_b, ln1_g, ln1_b, ln2_g, ln2_b)
    out, _ = run(x, shared)
    return out
